# revision 45
# baseline (speedup 1.0000x reference)
"""Trainium2 Bass kernel for BaichuanAttention (hidden=5120, 40 heads, b=2, s=2048).

Tensor-parallel over heads across 8 NeuronCores, all-bf16 datapath:

  Phase A: per-core QKV projection (5 heads) in bf16, qkv kept resident in
           SBUF (no DRAM round-trip).
  Phase B: flash-style causal attention computing scores TRANSPOSED
           (S^T = K_tile^T Q) so exp() output lands directly in the
           [k, q] layout attn@V needs -- no PE/DVE transposes of P.
           Row-sums via a ones-vector matmul; V transposed to token-major
           with a single XBAR DMA-transpose per head.
  Phase C: AllToAll of the (normalized) per-head attention outputs
           ([T, 640] bf16 per core, 16x less wire than reduce-scattering
           o_proj partials), then each core runs o_proj for its own
           512-token slice against the full W_o.
"""

import math
import sys

for _p in ("/opt/trn_rl_repo",):
    if _p not in sys.path:
        sys.path.insert(0, _p)

import numpy as np
import ml_dtypes

import concourse.bass as bass
import concourse.mybir as mybir
import concourse.tile as tile
from concourse import bacc, bass_utils

F32 = mybir.dt.float32
BF16 = mybir.dt.bfloat16
NPBF16 = ml_dtypes.bfloat16


class Cfg:
    def __init__(self, hidden=5120, n_heads=40, dh=128, B=2, S=2048, n_cores=8):
        self.hidden = hidden
        self.n_heads = n_heads
        self.dh = dh
        self.B = B
        self.S = S
        self.n_cores = n_cores
        assert dh == 128
        self.HL = n_heads // n_cores          # heads per core (5)
        self.F = 3 * self.HL * dh             # per-core packed qkv rows (1920)
        self.FO = self.HL * dh                # per-core attn feature width (640)
        self.T = B * S                        # total tokens (4096)
        self.KT = hidden // 128               # contraction tiles for qkv (40)
        self.FT = self.F // 128               # qkv feature tiles (15)
        self.SQT = S // 128                   # seq 128-tiles per batch (16)
        self.QC = S // 512                    # q chunks per batch (4)
        self.OC = hidden // 512               # o_proj output chunks (10)
        self.BLK = self.T // n_cores          # tokens per core after A2A (512)

    def key(self):
        return (self.hidden, self.n_heads, self.dh, self.B, self.S, self.n_cores)


def build_program(cfg: Cfg, mode: str):
    """mode: 'causal' (mask input ignored, causal skip), 'dense' (zero mask),
    'masked' (general additive mask, host passes mask^T * sqrt(dh))."""
    assert mode in ("causal", "dense", "masked")
    c = cfg
    nc = bacc.Bacc("TRN2", target_bir_lowering=False, debug=False,
                   num_devices=c.n_cores)
    GW = 5 if c.FT % 5 == 0 else 3            # phase A ft-group width
    NG = c.FT // GW
    assert c.FT % GW == 0
    CH0 = 512
    # X^T chunked on host: [128, T/CH, KT, CH] -> 16KB-contiguous DMA lines
    xg_t = nc.dram_tensor("xg", [128, c.T // CH0, c.KT, CH0], BF16,
                          kind="ExternalInput").ap()
    # W_pack shard, pre-grouped on host: [128, NG, KT, GW*128] so each group
    # half is one fully-contiguous-per-partition DMA
    wg = nc.dram_tensor("wg", [128, NG, c.KT, GW * 128], BF16,
                        kind="ExternalInput").ap()
    # full W_o^T, pre-grouped: [128, OC, KT, 512]
    wog = nc.dram_tensor("wog", [128, c.OC, c.KT, 512], BF16,
                         kind="ExternalInput").ap()
    maskt = None
    if mode == "masked":
        maskt = nc.dram_tensor("maskt", [c.S, c.S], F32,
                               kind="ExternalInput").ap()
    out_ext = nc.dram_tensor("out", [c.BLK, c.hidden], F32,
                             kind="ExternalOutput").ap()

    inv_sqrt_dh = 1.0 / math.sqrt(c.dh)
    NEG = -1.0e9
    HL, KT, FT, QC, SQT = c.HL, c.KT, c.FT, c.QC, c.SQT

    maskt_r = maskt.rearrange("(kt p) q -> p kt q", p=128) if maskt is not None else None
    CH = CH0                                  # phase A token chunk
    NH2 = CH // 512

    # per-batch A2A: each batch's attn output is exchanged separately so the
    # collective overlaps with the other batch's compute. Rank i owns token
    # rows [i*HB, (i+1)*HB) of each batch (HB = S / n_cores).
    HB = c.S // c.n_cores
    with tile.TileContext(nc) as tc:
        with tc.tile_pool(name="dram", bufs=1, space="DRAM") as dram:
            a2a_ins = [dram.tile([c.n_cores, c.FO, HB], BF16,
                                 tag=f"a2ai{b}", name=f"a2ai{b}")
                       for b in range(c.B)]
            a2a_outs = [dram.tile([c.n_cores, c.FO, HB], BF16,
                                  tag=f"a2ao{b}", name=f"a2ao{b}")
                        for b in range(c.B)]
            a2a_in_rs = [t.rearrange("blk (h p) t -> blk p h t", p=128)
                         for t in a2a_ins]
            a2a_out_rs = [t.rearrange("g (f p) t -> p (g f) t", p=128)
                          for t in a2a_outs]

            with tc.tile_pool(name="const", bufs=1) as cpool:
                ones_col = cpool.tile([128, 1], BF16, tag="onec")
                ones_row = cpool.tile([1, 128], BF16, tag="oner")
                with tc.tile_pool(name="ctmp", bufs=1) as tmp:
                    o32a = tmp.tile([128, 1], F32, tag="o32a")
                    nc.vector.memset(o32a[:], 1.0)
                    nc.vector.tensor_copy(ones_col[:], o32a[:])
                    o32b = tmp.tile([1, 128], F32, tag="o32b")
                    nc.vector.memset(o32b[:], 1.0)
                    nc.vector.tensor_copy(ones_row[:], o32b[:])
                cmask = None
                if mode == "causal":
                    # triangular 128x128 0/1 mask (bf16): 1 where q >= k
                    cmask = cpool.tile([128, 128], BF16, tag="cmask")
                    with tc.tile_pool(name="cmtmp", bufs=1) as cmt:
                        cm32 = cmt.tile([128, 128], F32, tag="cm32")
                        nc.gpsimd.memset(cm32[:], 1.0)
                        nc.gpsimd.affine_select(
                            out=cm32[:], in_=cm32[:],
                            compare_op=mybir.AluOpType.is_ge, fill=0.0,
                            base=0, pattern=[[1, 128]],
                            channel_multiplier=-1)
                        nc.vector.tensor_copy(cmask[:], cm32[:])

                with tc.tile_pool(name="qkvp", bufs=1) as qkvpool, \
                     tc.tile_pool(name="attp", bufs=1) as attpool, \
                     tc.tile_pool(name="vtkp", bufs=2) as vpool, \
                     tc.tile_pool(name="aw", bufs=2) as wpool, \
                     tc.tile_pool(name="ax", bufs=2) as xpool:
                    pending_w = {}
                    pending_x = {}
                    HKT = KT // 2

                    def load_w_half(g, half):
                        # [128, KT/2, GW*128] ko-half of one ft-group, split
                        # across both DMA queues
                        w_sb = wpool.tile([128, HKT, GW * 128], BF16, tag="w")
                        k0 = half * HKT
                        qk = HKT // 2
                        nc.sync.dma_start(w_sb[:, :qk], wg[:, g, k0:k0 + qk])
                        nc.scalar.dma_start(w_sb[:, qk:],
                                            wg[:, g, k0 + qk:k0 + HKT])
                        return w_sb

                    def load_x1(ci_g, kb):
                        # one [128, 8, CH] chunk, split across both DMA queues
                        x_sb = xpool.tile([128, 8, CH], BF16, tag="x",
                                          bufs=3)
                        nc.sync.dma_start(
                            x_sb[:, :4], xg_t[:, ci_g, kb * 8:kb * 8 + 4])
                        nc.scalar.dma_start(
                            x_sb[:, 4:], xg_t[:, ci_g, kb * 8 + 4:kb * 8 + 8])
                        return x_sb

                    for b in range(c.B):
                        qkv_sb = qkvpool.tile([128, FT, c.S], BF16, tag="qkv",
                                              name=f"qkv{b}")
                        attnT = attpool.tile([128, HL, c.S], BF16, tag="att",
                                             name=f"att{b}")
                        # ---------------- Phase A: QKV projection ---------
                        with tc.tile_pool(name=f"aps{b}", bufs=1,
                                          space="PSUM") as apsum:
                            for g in range(NG):
                                ft0 = g * GW
                                whs = pending_w.pop(b, None) if g == 0 \
                                    else None
                                if whs is None:
                                    whs = [load_w_half(g, 0),
                                           load_w_half(g, 1)]
                                for ci in range(c.S // CH):
                                    ci_g = (b * c.S + ci * CH) // CH
                                    pss = [apsum.tile([128, 512], F32,
                                                      tag=f"aps{u}",
                                                      name=f"aps{u}")
                                           for u in range(GW * NH2)]
                                    for kb in range(KT // 8):
                                        x_sb = None
                                        if g == 0 and ci == 0 and kb == 0:
                                            x_sb = pending_x.pop(b, None)
                                        if x_sb is None:
                                            x_sb = load_x1(ci_g, kb)
                                        for kj in range(8):
                                            ko = kb * 8 + kj
                                            w_sb = whs[ko // HKT]
                                            kl = ko % HKT
                                            for i in range(GW):
                                                for hf in range(NH2):
                                                    nc.tensor.matmul(
                                                        pss[i * NH2 + hf][:],
                                                        w_sb[:, kl,
                                                             i * 128:
                                                             (i + 1) * 128],
                                                        x_sb[:, kj,
                                                             hf * 512:
                                                             (hf + 1) * 512],
                                                        start=(ko == 0),
                                                        stop=(ko == KT - 1))
                                    for i in range(GW):
                                        for hf in range(NH2):
                                            o0 = ci * CH + hf * 512
                                            nc.vector.tensor_copy(
                                                qkv_sb[:, ft0 + i,
                                                       o0:o0 + 512],
                                                pss[i * NH2 + hf][:])

                        # ---------------- Phase B: attention --------------
                        with tc.tile_pool(name=f"bs{b}", bufs=2,
                                          space="PSUM") as spool, \
                             tc.tile_pool(name=f"bat{b}", bufs=2,
                                          space="PSUM") as batp, \
                             tc.tile_pool(name=f"brs{b}", bufs=2,
                                          space="PSUM") as rsbc, \
                             tc.tile_pool(name=f"bp{b}", bufs=9) as ppool, \
                             tc.tile_pool(name=f"bm{b}", bufs=3) as mpool, \
                             tc.tile_pool(name=f"bsm{b}", bufs=2) as smpool:
                            deferred = []

                            def flush():
                                while deferred:
                                    deferred.pop(0)()

                            for h in range(HL):
                                v_tok = vpool.tile([128, SQT, 128], BF16,
                                                   tag="vtok")
                                nc.sync.dma_start_transpose(
                                    v_tok[:], qkv_sb[:, 2 * HL + h, :])
                                for qc in range(QC):
                                    nkp = 2 * (qc + 1) if mode == "causal" \
                                        else SQT // 2
                                    nkt = 2 * nkp
                                    rs_ps = rsbc.tile([1, 512], F32,
                                                      tag="rsbc", name="rs")
                                    at_ps = batp.tile([128, 512], F32,
                                                      tag="at")
                                    pts = []

                                    def lo_of(kt):
                                        # first valid q column of k-tile kt
                                        if mode != "causal":
                                            return 0
                                        return max(0, kt * 128 - qc * 512)

                                    def emit_one_at(j, rs_ps=rs_ps,
                                                    at_ps=at_ps, pts=pts,
                                                    v_tok=v_tok, nkt=nkt,
                                                    lo_of=lo_of):
                                        for half in range(2):
                                            kt = 2 * j + half
                                            lo = lo_of(kt)
                                            nc.tensor.matmul(
                                                rs_ps[:, lo:], ones_col[:],
                                                pts[j][:, half, lo:],
                                                start=(kt == 0),
                                                stop=(kt == nkt - 1))
                                            nc.tensor.matmul(
                                                at_ps[:, lo:], v_tok[:, kt],
                                                pts[j][:, half, lo:],
                                                start=(kt == 0),
                                                stop=(kt == nkt - 1))

                                    for kp in range(nkp):
                                        s_ps = spool.tile([128, 2, 512], F32,
                                                          tag="s")
                                        p_sb = ppool.tile([128, 2, 512], BF16,
                                                          tag="p")
                                        for half in range(2):
                                            kt = 2 * kp + half
                                            lo = lo_of(kt)
                                            nc.tensor.matmul(
                                                s_ps[:, half, lo:],
                                                qkv_sb[:, HL + h,
                                                       kt * 128:(kt + 1) * 128],
                                                qkv_sb[:, h,
                                                       qc * 512 + lo:
                                                       (qc + 1) * 512],
                                                start=True, stop=True)
                                        if mode == "masked":
                                            m_sb = mpool.tile([128, 2, 512],
                                                              F32, tag="m")
                                            nc.sync.dma_start(
                                                m_sb[:],
                                                maskt_r[:, 2 * kp:2 * kp + 2,
                                                        qc * 512:(qc + 1) * 512])
                                            nc.vector.tensor_tensor(
                                                s_ps[:], s_ps[:], m_sb[:],
                                                mybir.AluOpType.add)
                                        for half in range(2):
                                            kt = 2 * kp + half
                                            lo = lo_of(kt)
                                            nc.scalar.activation(
                                                p_sb[:, half, lo:],
                                                s_ps[:, half, lo:],
                                                mybir.ActivationFunctionType.Exp,
                                                scale=inv_sqrt_dh)
                                            if mode == "causal" \
                                                    and kt >= 4 * qc:
                                                # zero the strict upper
                                                # triangle of the diagonal
                                                # block (post-exp 0/1 mask)
                                                nc.vector.tensor_tensor(
                                                    p_sb[:, half,
                                                         lo:lo + 128],
                                                    p_sb[:, half,
                                                         lo:lo + 128],
                                                    cmask[:],
                                                    mybir.AluOpType.mult)
                                        pts.append(p_sb)
                                        if kp == 1:
                                            flush()
                                        if kp >= 2:
                                            emit_one_at(kp - 2)
                                    if nkp == 1:
                                        flush()
                                    emit_one_at(nkp - 2)
                                    emit_one_at(nkp - 1)

                                    def finalize(h=h, qc=qc, rs_ps=rs_ps,
                                                 at_ps=at_ps, attnT=attnT,
                                                 b=b):
                                        # broadcast the row-sums to all 128
                                        # partitions first, then reciprocal
                                        # runs on 128 lanes instead of 1
                                        rsbf = smpool.tile([1, 512], BF16,
                                                           tag="rsbf")
                                        nc.scalar.copy(rsbf[:], rs_ps[:])
                                        bc_ps = rsbc.tile([128, 512], F32,
                                                          tag="rsbc",
                                                          name="bc")
                                        nc.tensor.matmul(bc_ps[:], ones_row[:],
                                                         rsbf[:], start=True,
                                                         stop=True)
                                        rqb_sb = smpool.tile([128, 512], F32,
                                                             tag="rqb")
                                        nc.vector.reciprocal_approx_fast(
                                            rqb_sb[:], bc_ps[:])
                                        nc.vector.tensor_tensor(
                                            attnT[:, h,
                                                  qc * 512:(qc + 1) * 512],
                                            at_ps[:], rqb_sb[:],
                                            mybir.AluOpType.mult)
                                        if qc == QC - 1:
                                            # ship this head's attn slices
                                            for j in range(c.n_cores):
                                                eng = nc.sync if j % 2 == 0 \
                                                    else nc.scalar
                                                eng.dma_start(
                                                    a2a_in_rs[b][j][:, h],
                                                    attnT[:, h,
                                                          j * HB:
                                                          (j + 1) * HB])

                                    deferred.append(finalize)
                                if h == 0 and qc == QC - 1 and b + 1 < c.B:
                                    # prefetch next batch's first W group and
                                    # first X chunk (within ring capacity --
                                    # deeper prefetch would block the queue)
                                    pending_w[b + 1] = [load_w_half(0, 0),
                                                        load_w_half(0, 1)]
                                    pending_x[b + 1] = \
                                        load_x1((b + 1) * c.S // CH, 0)
                            flush()
                        # per-batch AllToAll; b0's overlaps with b1 compute
                        nc.gpsimd.collective_compute(
                            "AllToAll",
                            mybir.AluOpType.bypass,
                            replica_groups=[list(range(c.n_cores))],
                            ins=[a2a_ins[b][:].opt()],
                            outs=[a2a_outs[b][:].opt()],
                        )

                # ---------------- Phase C: o_proj -------------------------
                with tc.tile_pool(name="catt", bufs=1) as cattp, \
                     tc.tile_pool(name="cwo", bufs=3) as wopool, \
                     tc.tile_pool(name="cout", bufs=4) as outpool, \
                     tc.tile_pool(name="cps", bufs=4, space="PSUM") as cpsum:
                    # my tokens: [0:HB) from batch 0, [HB:2*HB) from batch 1.
                    # Emit the first PRE ocs' batch-0 token-tiles before any
                    # batch-1 work so the second A2A hides under them; the
                    # W_o stream for those ocs is shared (bufs=PRE ring).
                    att_sb = cattp.tile([128, KT, c.BLK], BF16, tag="catt")
                    gstep = max(1, KT // 4)
                    TT = c.BLK // 128
                    h2 = TT // 2

                    def load_att(bb):
                        for g0 in range(0, KT, gstep):
                            g1 = min(g0 + gstep, KT)
                            eng = nc.sync if g0 % (2 * gstep) == 0 \
                                else nc.scalar
                            eng.dma_start(
                                att_sb[:, g0:g1, bb * HB:(bb + 1) * HB],
                                a2a_out_rs[bb][:, g0:g1])

                    def load_wo(oc):
                        wo_sb = wopool.tile([128, KT, 512], BF16, tag="wo")
                        hk = KT // 2
                        nc.sync.dma_start(wo_sb[:, :hk], wog[:, oc, :hk])
                        nc.scalar.dma_start(wo_sb[:, hk:], wog[:, oc, hk:])
                        return wo_sb

                    def emit_oc(oc, wo_sb, tts):
                        for tt in tts:
                            ps = cpsum.tile([128, 512], F32, tag="cps")
                            for ko in range(KT):
                                nc.tensor.matmul(
                                    ps[:],
                                    att_sb[:, ko, tt * 128:(tt + 1) * 128],
                                    wo_sb[:, ko],
                                    start=(ko == 0), stop=(ko == KT - 1))
                            o_sb = outpool.tile([128, 512], F32, tag="o")
                            nc.vector.tensor_copy(o_sb[:], ps[:])
                            nc.sync.dma_start(
                                out_ext[tt * 128:(tt + 1) * 128,
                                        oc * 512:(oc + 1) * 512],
                                o_sb[:])

                    PRE = min(3, c.OC) if c.B > 1 else 0
                    load_att(0)
                    wos = []
                    for oc in range(PRE):
                        wos.append(load_wo(oc))
                        emit_oc(oc, wos[oc], range(h2))
                    if c.B > 1:
                        load_att(1)
                    for oc in range(PRE):
                        emit_oc(oc, wos[oc], range(h2, TT))
                    for oc in range(PRE, c.OC):
                        wo_sb = load_wo(oc)
                        emit_oc(oc, wo_sb, range(TT))

    nc.compile()
    return nc


# --------------------------------------------------------------------------
_CACHE = {}


def _get_program(cfg: Cfg, mode: str):
    key = (cfg.key(), mode)
    if key not in _CACHE:
        _CACHE[key] = build_program(cfg, mode)
    return _CACHE[key]


def prepare_inputs(cfg: Cfg, hidden_states, attention_mask, W_pack, W_o):
    """Host-side shard + layout prep. Returns (mode, in_maps)."""
    c = cfg
    X = np.asarray(hidden_states, dtype=np.float32).reshape(c.T, c.hidden)
    # chunked X^T: [128, T/CH, KT, CH] (contiguous per-partition lines)
    CH = 512
    XG = np.ascontiguousarray(
        X.reshape(c.T // CH, CH, c.KT, 128).transpose(3, 0, 2, 1)
    ).astype(NPBF16)

    mask = np.asarray(attention_mask, dtype=np.float32).reshape(c.S, c.S)
    causal_ref = np.where(
        np.tril(np.ones((c.S, c.S), dtype=bool)), 0.0, -1e9
    ).astype(np.float32)
    if np.array_equal(mask, causal_ref):
        mode = "causal"
    elif not mask.any():
        mode = "dense"
    else:
        mode = "masked"

    W_pack = np.asarray(W_pack, dtype=np.float32)
    W_o = np.asarray(W_o, dtype=np.float32)
    H, KT, OC = c.hidden, c.KT, c.OC
    # full W_o^T grouped for phase C: [128, OC, KT, 512]
    wog = np.ascontiguousarray(
        W_o.T.reshape(KT, 128, OC, 512).transpose(1, 2, 0, 3)).astype(NPBF16)
    maskT = None
    if mode == "masked":
        maskT = np.ascontiguousarray(mask.T * math.sqrt(c.dh),
                                     dtype=np.float32)
    GW = 5 if c.FT % 5 == 0 else 3
    NG = c.FT // GW
    in_maps = []
    for g in range(c.n_cores):
        r0, r1 = g * c.FO, (g + 1) * c.FO
        wq = W_pack[r0:r1]
        wk = W_pack[H + r0:H + r1]
        wv = W_pack[2 * H + r0:2 * H + r1]
        wqkvT = np.concatenate([wq, wk, wv], axis=0).T   # [H, F]
        # grouped for phase A: [128, NG, KT, GW*128]
        wgg = np.ascontiguousarray(
            wqkvT.reshape(KT, 128, NG, GW * 128).transpose(1, 2, 0, 3)
        ).astype(NPBF16)
        m = {"xg": XG, "wg": wgg, "wog": wog}
        if mode == "masked":
            m["maskt"] = maskT
        in_maps.append(m)
    return mode, in_maps


def assemble_output(cfg: Cfg, results):
    c = cfg
    HB = c.S // c.n_cores
    full = np.empty((c.T, c.hidden), dtype=np.float32)
    for g in range(c.n_cores):
        o = results[g]["out"]
        for b in range(c.B):
            full[b * c.S + g * HB:b * c.S + (g + 1) * HB] = \
                o[b * HB:(b + 1) * HB]
    return full.reshape(c.B, c.S, c.hidden)


def kernel(hidden_states, attention_mask, W_pack, W_o):
    cfg = Cfg()
    mode, in_maps = prepare_inputs(cfg, hidden_states, attention_mask,
                                   W_pack, W_o)
    nc = _get_program(cfg, mode)
    res = bass_utils.run_bass_kernel_spmd(nc, in_maps,
                                          list(range(cfg.n_cores)))
    return assemble_output(cfg, res.results)
